# revision 1
# baseline (speedup 1.0000x reference)
"""Trainium2 Bass kernel for nn_BioGNN (3-layer GAT + mean-pool + linear head).

8-core SPMD strategy:
  - Nodes sharded into 8 contiguous ranges (6250/core, padded to 6272=49*128).
  - Per layer: dense transform on PE (augmented weights also produce per-node
    attention terms e_src/e_dst as extra columns); AllGather the per-node rows
    [hlin(256)|e_src(4)|e_dst(4)]; edge phase gathers src rows by indirect DMA,
    computes ex=exp(leakyrelu(e_src+e_dst)) per edge, and scatter-accumulates
    Sum(ex*h_src) and Sum(ex) per dst node via one-hot matmuls on PE; epilogue
    normalizes, adds bias, applies ELU (layers 1-2).
  - Pooling via one-hot(batch) matmul + AllReduce + linear head.

Softmax skips the segment-max shift (logits are O(10), exp is safe in fp32 and
alpha is shift-invariant).
"""
import sys

for _p in ("/opt/trn_rl_repo", "/root/.axon_site/_ro/trn_rl_repo"):
    if _p not in sys.path:
        sys.path.insert(0, _p)

import numpy as np

import concourse.bass as bass
import concourse.tile as tile
from concourse import bacc, mybir
from concourse.bass import IndirectOffsetOnAxis
from concourse.bass_utils import run_bass_kernel_spmd
from concourse.library_config import mlp as mlp_lib

P = 128
NCORES = 8
FDT = mybir.dt.float32
IDT = mybir.dt.int32
I16 = mybir.dt.int16
AF = mybir.ActivationFunctionType
ALU = mybir.AluOpType

# problem config (hardcoded per spec); tests may build scaled-down variants
CFG = dict(N=50000, G=64, IN=128, HID=64, H=4, OUT=10)
ABLATE = set()  # perf-ablation switches for modeling only
ROWP_OF = {264: 320}  # gather row padded to a 256B multiple (f32)


def build_program(TBS, cfg=CFG):
    """Build the SPMD program. TBS: per-block tile counts (len NB), identical
    across cores."""
    N, G, IN, HID, H, OUTF = (cfg["N"], cfg["G"], cfg["IN"], cfg["HID"],
                              cfg["H"], cfg["OUT"])
    F = H * HID
    ROW = F + 8
    ROWP = ROWP_OF[ROW]
    NSH = N // NCORES
    NB = (NSH + P - 1) // P
    NSHP = NB * P
    HALF = NCORES * NSHP // 2
    assert len(TBS) == NB and all(len(t) == 2 for t in TBS)
    TBSUM = [lo + hi for lo, hi in TBS]
    TT = sum(TBSUM)
    KT = F // P                    # K-tiles for layers 2-3 (2)

    nc = bacc.Bacc("TRN2", target_bir_lowering=False, debug=False,
                   num_devices=NCORES)

    # ---- I/O ----
    xT = nc.dram_tensor("xT", [IN, NSHP], FDT, kind="ExternalInput")
    idx16 = nc.dram_tensor("idx16", [P, 8 * TT], I16, kind="ExternalInput")
    dstrow = nc.dram_tensor("dstrow", [1, TT * P], FDT, kind="ExternalInput")
    iotap_in = nc.dram_tensor("iotap", [P, 1], FDT, kind="ExternalInput")
    dstloc = nc.dram_tensor("dstloc", [P, TT], FDT, kind="ExternalInput")
    batchloc = nc.dram_tensor("batchloc", [P, NB], FDT, kind="ExternalInput")
    iota_in = nc.dram_tensor("iota", [P, P], FDT, kind="ExternalInput")
    ident_in = nc.dram_tensor("ident", [P, P], FDT, kind="ExternalInput")
    wts = [nc.dram_tensor(f"wt{l}", [IN if l == 1 else F, ROW], FDT,
                          kind="ExternalInput") for l in (1, 2, 3)]
    breps = [nc.dram_tensor(f"brep{l}", [P, F], FDT, kind="ExternalInput")
             for l in (1, 2, 3)]
    wlt = nc.dram_tensor("wlt", [F, OUTF], FDT, kind="ExternalInput")
    blrep = nc.dram_tensor("blrep", [G, OUTF], FDT, kind="ExternalInput")
    invcnt = nc.dram_tensor("invcnt", [G, 1], FDT, kind="ExternalInput")
    out_ext = nc.dram_tensor("out", [G, OUTF], FDT, kind="ExternalOutput")

    # ---- internal DRAM ----
    hrow_own = nc.dram_tensor("hrow_own", [NSHP, ROWP], FDT)
    hrow_full = nc.dram_tensor("hrow_full", [NCORES * NSHP, ROWP], FDT,
                               addr_space="Shared")
    hT_own = [nc.dram_tensor(f"hT_own{l}", [F, NSHP], FDT) for l in (1, 2)]
    pool_own = nc.dram_tensor("pool_own", [G, F], FDT)
    pool_full = nc.dram_tensor("pool_full", [G, F], FDT, addr_space="Shared")

    with tile.TileContext(nc) as tc:
        with (
            tc.tile_pool(name="const", bufs=1) as cpool,
            tc.tile_pool(name="wpool", bufs=2) as wpool,
            tc.tile_pool(name="sb", bufs=3) as pool,
            tc.tile_pool(name="gpool", bufs=2) as gpool,
            tc.tile_pool(name="ps", bufs=2, space="PSUM") as pspool,
            tc.tile_pool(name="pspool1", bufs=1, space="PSUM") as pspool1,
        ):
            # resident constants
            iota_sb = cpool.tile([P, P], FDT)
            nc.sync.dma_start(iota_sb[:], iota_in[:])
            ident_sb = cpool.tile([P, P], FDT)
            nc.sync.dma_start(ident_sb[:], ident_in[:])
            idx16_sb = cpool.tile([P, 8 * TT], I16)
            nc.sync.dma_start(idx16_sb[:], idx16[:])
            iotap_sb = cpool.tile([P, 1], FDT)
            nc.sync.dma_start(iotap_sb[:], iotap_in[:])
            dstloc_sb = cpool.tile([P, TT], FDT)
            nc.sync.dma_start(dstloc_sb[:], dstloc[:])
            batchloc_sb = cpool.tile([P, NB], FDT)
            nc.sync.dma_start(batchloc_sb[:], batchloc[:])
            nc.gpsimd.load_library(mlp_lib)
            zcol_sb = cpool.tile([P, ROWP - ROW], FDT)
            nc.vector.memset(zcol_sb[:], 0.0)
            # zero the gather-row pad columns once (persist across layers)
            for b in range(NB):
                nc.sync.dma_start(hrow_own[b * P:(b + 1) * P, ROW:], zcol_sb[:])
            tc.strict_bb_all_engine_barrier()

            pool_ps = pspool1.tile([G, F], mybir.dt.float32, tag="pool")

            for layer in (1, 2, 3):
                kt = 1 if layer == 1 else KT
                wt_sb = []
                for k in range(kt):
                    w = wpool.tile([P, ROW], FDT, tag=f"wt{k}")
                    nc.sync.dma_start(w[:], wts[layer - 1][k * P:(k + 1) * P, :])
                    wt_sb.append(w)
                brep_sb = wpool.tile([P, F], FDT, tag="brep")
                nc.sync.dma_start(brep_sb[:], breps[layer - 1][:])

                # ---- phase A: dense + write hrow_own ----
                for b in range(NB):
                    hlin_ps = pspool.tile([P, ROW], mybir.dt.float32, tag="mm")
                    for k in range(kt):
                        lt = pool.tile([P, P], FDT, tag="lhsT")
                        if layer == 1:
                            nc.scalar.dma_start(lt[:], xT[:, b * P:(b + 1) * P])
                        else:
                            nc.scalar.dma_start(
                                lt[:],
                                hT_own[layer - 2][k * P:(k + 1) * P,
                                                  b * P:(b + 1) * P])
                        lhsT = lt[:]
                        nc.tensor.matmul(hlin_ps[:], lhsT=lhsT, rhs=wt_sb[k][:],
                                         start=(k == 0), stop=(k == kt - 1))
                    hrow_sb = pool.tile([P, ROW], FDT, tag="hrow")
                    nc.vector.tensor_copy(hrow_sb[:], hlin_ps[:])
                    nc.sync.dma_start(hrow_own[b * P:(b + 1) * P, :ROW], hrow_sb[:])

                # ---- phase B: AllGather rows ----
                nc.gpsimd.collective_compute(
                    "AllGather", ALU.bypass,
                    ins=[hrow_own[:]], outs=[hrow_full[:]],
                    replica_groups=[list(range(NCORES))],
                )

                # ---- phase C: edge phase ----
                t0 = 0
                for b in range(NB):
                    Tb = TBSUM[b]
                    numer_ps = pspool.tile([P, F], mybir.dt.float32, tag="mm")
                    dn_ps = pspool.tile([P, 8 * Tb], mybir.dt.float32,
                                        tag="denom")
                    edst_ps = dn_ps[:, :4 * Tb]
                    denom_ps = dn_ps[:, 4 * Tb:]
                    gath = gpool.tile([P, Tb * ROWP], FDT, tag="gath")
                    goff = 0
                    for half in (0, 1):
                        Tg = TBS[b][half]
                        if Tg == 0:
                            continue
                        if "gather" in ABLATE:
                            if half == 0:
                                nc.gpsimd.memset(gath[:], 0.0)
                            goff += Tg
                            continue
                        # dma_gather crashes above 1024 idxs/call: chunk <=8 tiles
                        done = 0
                        while done < Tg:
                            ck = min(8, Tg - done)
                            o = goff + done
                            nc.gpsimd.dma_gather(
                                out_ap=gath[:, o * ROWP:(o + ck) * ROWP]
                                    .rearrange("p (t e) -> p t e", e=ROWP),
                                in_ap=hrow_full[half * HALF:(half + 1) * HALF, :],
                                idxs_ap=idx16_sb[:, 8 * (t0 + o):8 * (t0 + o + ck)],
                                num_idxs=ck * P,
                                num_idxs_reg=ck * P,
                                elem_size=ROWP,
                            )
                            done += ck
                        goff += Tg
                    # e_dst expansion: PT[m,e] one-hot, edst_g = PT.T @ edb
                    dstrow_sb = pool.tile([1, Tb * P], FDT, tag="dstrow")
                    nc.scalar.dma_start(dstrow_sb[:],
                                      dstrow[0:1, t0 * P:(t0 + Tb) * P])
                    drep = gpool.tile([P, Tb * P], FDT, tag="drep")
                    if "pbcast" in ABLATE:
                        nc.gpsimd.memset(drep[:], 0.0)
                    if "pbcast" not in ABLATE:
                        nc.gpsimd.partition_broadcast(drep[:], dstrow_sb[:])
                    ptall = gpool.tile([P, Tb * P], FDT, tag="ptall")
                    if "ptbuild" in ABLATE:
                        nc.gpsimd.memset(ptall[:], 0.0)
                    if "ptbuild" not in ABLATE:
                        nc.vector.tensor_scalar(
                            out=ptall[:], in0=drep[:], scalar1=iotap_sb[:, 0:1],
                            scalar2=None, op0=ALU.is_equal)
                    edb = pool.tile([P, 4], FDT, tag="edb")
                    nc.scalar.dma_start(edb[:],
                                      hrow_own[b * P:(b + 1) * P, F + 4:F + 8])
                    if "edstmm" in ABLATE:
                        nc.vector.memset(dn_ps[:, :4], 0.0)
                    if "edstmm" not in ABLATE:
                        for t in range(Tb):
                            nc.tensor.matmul(edst_ps[:, 4 * t:4 * t + 4],
                                             lhsT=ptall[:, t * P:(t + 1) * P],
                                             rhs=edb[:], start=True, stop=True)
                    # logits = e_src(gathered) + e_dst(expanded)
                    lg = pool.tile([P, 4 * Tb], FDT, tag="lg")
                    nc.vector.tensor_tensor(
                        out=lg[:].rearrange("p (t f) -> p t f", f=4),
                        in0=gath[:].rearrange("p (t r) -> p t r", r=ROWP)[:, :, F:F + 4],
                        in1=edst_ps.rearrange("p (t f) -> p t f", f=4),
                        op=ALU.add)
                    # leaky relu (0.2) then exp
                    lr = pool.tile([P, 4 * Tb], FDT, tag="lr")
                    nc.vector.scalar_tensor_tensor(
                        out=lr[:], in0=lg[:], scalar=0.2, in1=lg[:],
                        op0=ALU.mult, op1=ALU.max)
                    ex = pool.tile([P, 4 * Tb], FDT, tag="ex")
                    nc.scalar.activation(ex[:], lr[:], AF.Exp)

                    for t in range(Tb):
                        pmat = pool.tile([P, P], FDT, tag="pmat")
                        if "pbuild" in ABLATE:
                            nc.gpsimd.memset(pmat[:], 0.0)
                        if "pbuild" not in ABLATE:
                            nc.vector.tensor_scalar(
                                out=pmat[:], in0=iota_sb[:],
                                scalar1=dstloc_sb[:, t0 + t:t0 + t + 1],
                                scalar2=None, op0=ALU.is_equal)
                        msg = pool.tile([P, F], FDT, tag="msg")
                        if "muls" in ABLATE:
                            nc.gpsimd.memset(msg[:], 0.0)
                        if "muls" not in ABLATE:
                            for h in range(H):
                                nc.vector.tensor_scalar_mul(
                                    msg[:, h * HID:(h + 1) * HID],
                                    gath[:, t * ROWP + h * HID:t * ROWP + (h + 1) * HID],
                                    ex[:, 4 * t + h:4 * t + h + 1])
                        if "scatter" in ABLATE and t == 0:
                            nc.vector.memset(numer_ps[:], 0.0)
                            nc.vector.memset(dn_ps[:], 0.0)
                        if "scatter" not in ABLATE:
                            nc.tensor.matmul(numer_ps[:], lhsT=pmat[:], rhs=msg[:],
                                             start=(t == 0), stop=(t == Tb - 1))
                            nc.tensor.matmul(denom_ps[:, 4 * t:4 * t + 4][:],
                                             lhsT=pmat[:], rhs=ex[:, 4 * t:4 * t + 4],
                                             start=True, stop=True)
                    # fold Tb denom groups into [P,4]
                    dsum = pool.tile([P, 4], FDT, tag="dsum")
                    nc.vector.reduce_sum(
                        out=dsum[:],
                        in_=denom_ps.rearrange("p (t f) -> p f t", f=4),
                        axis=mybir.AxisListType.X)
                    # guard pad nodes (zero in-degree): denom=0 -> inf -> NaN
                    nc.vector.tensor_scalar_max(dsum[:], dsum[:], 1e-12)
                    rec = pool.tile([P, 4], FDT, tag="rec")
                    nc.vector.reciprocal(rec[:], dsum[:])
                    # y = numer/denom + b
                    y = pool.tile([P, F], FDT, tag="y")
                    for h in range(H):
                        nc.vector.tensor_scalar_mul(
                            y[:, h * HID:(h + 1) * HID],
                            numer_ps[:, h * HID:(h + 1) * HID],
                            rec[:, h:h + 1])
                    nc.vector.tensor_tensor(out=y[:], in0=y[:], in1=brep_sb[:],
                                            op=ALU.add)
                    if layer < 3:
                        # ELU: relu(y) + exp(min(y,0)) - 1
                        mn = pool.tile([P, F], FDT, tag="mn")
                        nc.vector.tensor_scalar_min(mn[:], y[:], 0.0)
                        eu = pool.tile([P, F], FDT, tag="eu")
                        nc.scalar.activation(eu[:], mn[:], AF.Exp)
                        rl = pool.tile([P, F], FDT, tag="rl")
                        nc.scalar.activation(rl[:], y[:], AF.Relu)
                        hv = pool.tile([P, F], FDT, tag="hv")
                        nc.vector.scalar_tensor_tensor(
                            out=hv[:], in0=eu[:], scalar=-1.0, in1=rl[:],
                            op0=ALU.add, op1=ALU.add)
                        # transpose -> hT_own for next layer's dense phase
                        for k in range(KT):
                            tp = pspool.tile([P, P], mybir.dt.float32, tag="tp")
                            nc.tensor.transpose(tp[:], hv[:, k * P:(k + 1) * P],
                                                ident_sb[:])
                            tps = pool.tile([P, P], FDT, tag="tps")
                            nc.vector.tensor_copy(tps[:], tp[:])
                            nc.sync.dma_start(
                                hT_own[layer - 1][k * P:(k + 1) * P,
                                                  b * P:(b + 1) * P], tps[:])
                    else:
                        # pooling accumulation
                        bmat = pool.tile([P, G], FDT, tag="bmat")
                        nc.vector.tensor_scalar(
                            out=bmat[:], in0=iota_sb[:, :G],
                            scalar1=batchloc_sb[:, b:b + 1],
                            scalar2=None, op0=ALU.is_equal)
                        nc.tensor.matmul(pool_ps[:], lhsT=bmat[:], rhs=y[:],
                                         start=(b == 0), stop=(b == NB - 1))
                    t0 += Tb
                # fence between layers: hrow/hT buffers are reused
                tc.strict_bb_all_engine_barrier()

            # ---- final: pool -> AllReduce -> mean -> linear ----
            pool_sb = pool.tile([G, F], FDT, tag="poolsb")
            nc.vector.tensor_copy(pool_sb[:], pool_ps[:])
            nc.sync.dma_start(pool_own[:], pool_sb[:])
            nc.gpsimd.collective_compute(
                "AllReduce", ALU.add,
                ins=[pool_own[:]], outs=[pool_full[:]],
                replica_groups=[list(range(NCORES))],
            )
            invcnt_sb = cpool.tile([G, 1], FDT)
            nc.sync.dma_start(invcnt_sb[:], invcnt[:])
            wlt_sb = []
            for k in range(KT):
                w = cpool.tile([P, OUTF], FDT)
                nc.sync.dma_start(w[:], wlt[k * P:(k + 1) * P, :])
                wlt_sb.append(w)
            blrep_sb = cpool.tile([G, OUTF], FDT)
            nc.sync.dma_start(blrep_sb[:], blrep[:])

            pooled = pool.tile([G, F], FDT, tag="pooled")
            nc.sync.dma_start(pooled[:], pool_full[:])
            mean = pool.tile([G, F], FDT, tag="mean")
            nc.vector.tensor_scalar_mul(mean[:], pooled[:], invcnt_sb[:])
            fin_ps = pspool.tile([G, OUTF], mybir.dt.float32, tag="tp")
            for k in range(KT):
                ptp = pspool.tile([P, G], mybir.dt.float32, tag="tp")
                nc.tensor.transpose(ptp[:], mean[:, k * P:(k + 1) * P],
                                    ident_sb[:G, :G])
                ptps = pool.tile([P, G], FDT, tag="ptps")
                nc.vector.tensor_copy(ptps[:], ptp[:])
                nc.tensor.matmul(fin_ps[:], lhsT=ptps[:], rhs=wlt_sb[k][:],
                                 start=(k == 0), stop=(k == KT - 1))
            outv = pool.tile([G, OUTF], FDT, tag="outv")
            nc.vector.tensor_tensor(out=outv[:], in0=fin_ps[:], in1=blrep_sb[:],
                                    op=ALU.add)
            nc.sync.dma_start(out_ext[:], outv[:])

    nc.compile()
    return nc


def preprocess(x, edge_index, batch, params, cfg=CFG):
    """Host-side index preprocessing + param packing -> (TBS, in_maps)."""
    N, G, IN, HID, H, OUTF = (cfg["N"], cfg["G"], cfg["IN"], cfg["HID"],
                              cfg["H"], cfg["OUT"])
    F = H * HID
    NSH = N // NCORES
    NB = (NSH + P - 1) // P
    NSHP = NB * P

    HALF = NCORES * NSHP // 2
    src = np.concatenate([np.asarray(edge_index[0]), np.arange(N)]).astype(np.int64)
    dst = np.concatenate([np.asarray(edge_index[1]), np.arange(N)]).astype(np.int64)
    batch = np.asarray(batch).astype(np.int64)

    def remap(nodes):
        return (nodes // NSH) * NSHP + nodes % NSH

    core_of = dst // NSH
    tiles_lo = np.zeros((NCORES, NB), np.int64)
    tiles_hi = np.zeros((NCORES, NB), np.int64)
    per_core = []
    for c in range(NCORES):
        m = core_of == c
        s_c, d_c = remap(src[m]), dst[m] - c * NSH
        # sort by (block, half, dst) so each block is lo-group then hi-group
        half_c = (s_c >= HALF).astype(np.int64)
        blk = d_c // P
        order = np.lexsort((d_c, half_c, blk))
        s_c, d_c, half_c, blk = s_c[order], d_c[order], half_c[order], blk[order]
        cnt_lo = np.bincount(blk[half_c == 0], minlength=NB)
        cnt_hi = np.bincount(blk[half_c == 1], minlength=NB)
        tiles_lo[c] = (cnt_lo + P - 1) // P
        tiles_hi[c] = (cnt_hi + P - 1) // P
        per_core.append((s_c, d_c, half_c, blk, cnt_lo, cnt_hi))
    TBS = [(int(max(tiles_lo[:, b].max(), 1)), int(tiles_hi[:, b].max()))
           for b in range(NB)]
    TBSUM = [lo + hi for lo, hi in TBS]
    TT = sum(TBSUM)
    tb0 = np.cumsum([0] + TBSUM[:-1])
    tbhi0 = [tb0[b] + TBS[b][0] for b in range(NB)]  # first hi tile per block

    W = {k: np.asarray(v, np.float64) for k, v in params.items()}
    wt_aug = {}
    for l in (1, 2, 3):
        Wl = W[f"W{l}"]
        asrc, adst = W[f"a_src{l}"], W[f"a_dst{l}"]
        fin = Wl.shape[1]
        Ablk_s = np.zeros((F, H))
        Ablk_d = np.zeros((F, H))
        for h in range(H):
            Ablk_s[h * HID:(h + 1) * HID, h] = asrc[h]
            Ablk_d[h * HID:(h + 1) * HID, h] = adst[h]
        wt_aug[l] = np.concatenate(
            [Wl.T, Wl.T @ Ablk_s, Wl.T @ Ablk_d], axis=1).astype(np.float32)

    counts = np.bincount(batch, minlength=G).astype(np.float64)
    invcnt = (1.0 / np.maximum(counts, 1.0)).astype(np.float32)[:, None]
    iota = np.tile(np.arange(P, dtype=np.float32), (P, 1))
    ident = np.eye(P, dtype=np.float32)

    in_maps = []
    xarr = np.asarray(x)
    for c in range(NCORES):
        s_c, d_c, half_c, blk, cnt_lo, cnt_hi = per_core[c]
        # slot index within the (block, half) group
        grp_key = blk * 2 + half_c
        grp_cnt = np.bincount(grp_key, minlength=2 * NB)
        grp_start = np.concatenate([[0], np.cumsum(grp_cnt)[:-1]])
        pos_in_grp = np.arange(len(d_c)) - grp_start[grp_key]
        grp_t0 = np.where(half_c == 0, tb0[blk], np.asarray(tbhi0)[blk])
        t_idx = (grp_t0 + pos_in_grp // P).astype(np.int64)
        p_idx = (pos_in_grp % P).astype(np.int64)

        dstloc = np.full((P, TT), -1.0, np.float32)
        dstloc[p_idx, t_idx] = (d_c - blk * P).astype(np.float32)
        dstrow = np.ascontiguousarray(dstloc.T).reshape(1, TT * P)

        # int16 wrapped gather indices: slot j of tile t -> column 8*t + j//16,
        # partitions p with p%16 == j%16 (replicated across the 8 groups)
        idxflat = np.zeros(TT * P, np.int16)
        idxflat[t_idx * P + p_idx] = (s_c - half_c * HALF).astype(np.int16)
        idx16 = np.ascontiguousarray(
            np.tile(idxflat.reshape(TT * 8, 16).T, (8, 1))).astype(np.int16)

        batchloc = np.full(NSHP, -1.0, np.float32)
        batchloc[:NSH] = batch[c * NSH:(c + 1) * NSH]
        batchloc = np.ascontiguousarray(batchloc.reshape(NB, P).T)

        xT_own = np.zeros((IN, NSHP), np.float32)
        xT_own[:, :NSH] = xarr[c * NSH:(c + 1) * NSH].T

        in_maps.append(dict(
            xT=xT_own, idx16=idx16, dstrow=dstrow, dstloc=dstloc,
            iotap=np.arange(P, dtype=np.float32)[:, None],
            batchloc=batchloc, iota=iota, ident=ident,
            wt1=wt_aug[1], wt2=wt_aug[2], wt3=wt_aug[3],
            brep1=np.tile(W["b1"].astype(np.float32), (P, 1)),
            brep2=np.tile(W["b2"].astype(np.float32), (P, 1)),
            brep3=np.tile(W["b3"].astype(np.float32), (P, 1)),
            wlt=np.ascontiguousarray(W["Wl"].T.astype(np.float32)),
            blrep=np.tile(W["bl"].astype(np.float32), (G, 1)),
            invcnt=invcnt,
        ))
    return TBS, in_maps


def kernel(**inputs):
    x = inputs.pop("x")
    edge_index = inputs.pop("edge_index")
    batch = inputs.pop("batch")
    TBS, in_maps = preprocess(x, edge_index, batch, inputs)
    nc = build_program(TBS)
    res = run_bass_kernel_spmd(nc, in_maps, list(range(NCORES)))
    return np.asarray(res.results[0]["out"], np.float32)



# revision 14
# speedup vs baseline: 2.2529x; 2.2529x over previous
"""Trainium2 Bass kernel for nn_BioGNN (3-layer GAT + mean-pool + linear head).

8-core SPMD strategy:
  - Nodes sharded into 8 contiguous ranges (6250/core, padded to 6272=49*128).
  - Per layer: dense transform on PE (augmented weights also produce per-node
    attention terms e_src/e_dst as extra columns); AllGather the per-node bf16
    rows [h(256)|e_src(4)|e_dst(4)|pad->384]; edge phase gathers src rows by
    indirect DMA (768B/row), computes ex=exp(leakyrelu(e_src+e_dst)) per edge,
    and scatter-accumulates [Sum(ex*h_src) | Sum(ex)] per dst node via one
    one-hot matmul per tile on PE; epilogue normalizes, adds bias, applies ELU
    (layers 1-2).
  - Pooling via one-hot(batch) matmul + AllReduce + linear head (fp32).

Most edge-phase tensors are bf16 (tolerance is 2e-2; bf16 keeps ~3e-3).
Per-block DVE ops are batched via free-dim-broadcast access patterns.
Softmax skips the segment-max shift (logits are O(10), exp is safe in bf16's
range and alpha is shift-invariant).
"""
import os
import sys

for _p in ("/opt/trn_rl_repo", "/root/.axon_site/_ro/trn_rl_repo"):
    if _p not in sys.path:
        sys.path.insert(0, _p)

import numpy as np
import ml_dtypes

import concourse.bass as bass
import concourse.tile as tile
from concourse import bacc, mybir
from concourse.bass_utils import run_bass_kernel_spmd
from concourse.library_config import mlp as mlp_lib

P = 128
NCORES = 8
F32 = mybir.dt.float32
BF = mybir.dt.bfloat16
U8 = mybir.dt.uint8
F8 = mybir.dt.float8e4
I16 = mybir.dt.int16
I32 = mybir.dt.int32
AF = mybir.ActivationFunctionType
ALU = mybir.AluOpType
BF_NP = ml_dtypes.bfloat16
F8_NP = ml_dtypes.float8_e4m3

# problem config (hardcoded per spec); tests may build scaled-down variants
CFG = dict(N=50000, G=64, IN=128, HID=64, H=4, OUT=10)
ABLATE = set(os.environ.get("KABLATE", "").split(",")) - {""}
ROW = 264                      # h(256) | e_src(4) | e_dst(4)
ROWP = 384                     # bf16 row padded to a 256B multiple (768B)


def build_program(TBS, cfg=CFG, sim_single=False):
    """Build the SPMD program. TBS: per-block tile counts (len NB), identical
    across cores. sim_single=True builds a 1-device timing model (collectives
    replaced by local copies) for TimelineSim analysis only."""
    N, G, IN, HID, H, OUTF = (cfg["N"], cfg["G"], cfg["IN"], cfg["HID"],
                              cfg["H"], cfg["OUT"])
    F = H * HID
    NSH = N // NCORES
    NB = (NSH + P - 1) // P
    NSHP = NB * P
    HALF = NCORES * NSHP // 2
    assert len(TBS) == NB and all(len(t) == 2 for t in TBS)
    TBSUM = [lo + hi for lo, hi in TBS]
    TT = sum(TBSUM)
    TBMAX = max(TBSUM)
    NCALLS = sum((tg + 7) // 8 for pair in TBS for tg in pair)
    KT = F // P                    # K-tiles for layers 2-3 (2)

    nc = bacc.Bacc("TRN2", target_bir_lowering=False, debug=False,
                   num_devices=1 if sim_single else NCORES)

    # ---- I/O ----
    xT = nc.dram_tensor("xT", [IN, NSHP], BF, kind="ExternalInput")
    idx16 = nc.dram_tensor("idx16", [P, 8 * TT], I16, kind="ExternalInput")
    ptall_in = nc.dram_tensor("ptall8", [P, TT * P], F8, kind="ExternalInput")
    dstloc = nc.dram_tensor("dstloc", [P, TT], F32, kind="ExternalInput")
    gcnt = nc.dram_tensor("gcnt", [1, NCALLS], I32, kind="ExternalInput")
    iotap_in = nc.dram_tensor("iotap", [P, 1], F32, kind="ExternalInput")
    batchloc = nc.dram_tensor("batchloc", [P, NB], F32, kind="ExternalInput")
    iota_in = nc.dram_tensor("iota", [P, P], BF, kind="ExternalInput")
    identb_in = nc.dram_tensor("identb", [P, P], BF, kind="ExternalInput")
    identf_in = nc.dram_tensor("identf", [G, G], F32, kind="ExternalInput")
    wts = [nc.dram_tensor(f"wt{l}", [IN if l == 1 else F, ROW], BF,
                          kind="ExternalInput") for l in (1, 2, 3)]
    breps = [nc.dram_tensor(f"brep{l}", [P, F], BF, kind="ExternalInput")
             for l in (1, 2, 3)]
    wlt = nc.dram_tensor("wlt", [F, OUTF], F32, kind="ExternalInput")
    blrep = nc.dram_tensor("blrep", [G, OUTF], F32, kind="ExternalInput")
    invcnt = nc.dram_tensor("invcnt", [G, 1], F32, kind="ExternalInput")
    out_ext = nc.dram_tensor("out", [G, OUTF], F32, kind="ExternalOutput")

    # ---- internal DRAM ----
    shr = {} if sim_single else dict(addr_space="Shared")
    hrow_own = nc.dram_tensor("hrow_own", [NSHP, ROWP], BF)
    hrow_full = nc.dram_tensor("hrow_full", [NCORES * NSHP, ROWP], BF, **shr)
    hT_own = [nc.dram_tensor(f"hT_own{l}", [F, NSHP], BF) for l in (1, 2)]
    pool_own = nc.dram_tensor("pool_own", [G, F], F32)
    pool_full = nc.dram_tensor("pool_full", [G, F], F32, **shr)

    with tile.TileContext(nc) as tc:
        with (
            tc.tile_pool(name="const", bufs=1) as cpool,
            tc.tile_pool(name="wpool", bufs=2) as wpool,
            tc.tile_pool(name="sb", bufs=3) as pool,
            tc.tile_pool(name="gpool", bufs=2) as gpool,
            tc.tile_pool(name="ps", bufs=2, space="PSUM") as pspool,
            tc.tile_pool(name="pspool1", bufs=1, space="PSUM") as pspool1,
        ):
            # resident constants
            iota_sb = cpool.tile([P, P], BF)
            nc.sync.dma_start(iota_sb[:], iota_in[:])
            identb_sb = cpool.tile([P, P], BF)
            nc.sync.dma_start(identb_sb[:], identb_in[:])
            identf_sb = cpool.tile([G, G], F32)
            nc.sync.dma_start(identf_sb[:], identf_in[:])
            idx16_sb = cpool.tile([P, 8 * TT], I16)
            nc.sync.dma_start(idx16_sb[:], idx16[:])
            iotap_sb = cpool.tile([P, 1], F32)
            nc.sync.dma_start(iotap_sb[:], iotap_in[:])
            dstloc_sb = cpool.tile([P, TT], F32)
            nc.sync.dma_start(dstloc_sb[:], dstloc[:])
            batchloc_sb = cpool.tile([P, NB], F32)
            nc.sync.dma_start(batchloc_sb[:], batchloc[:])
            gcnt_sb = cpool.tile([1, NCALLS], I32)
            nc.sync.dma_start(gcnt_sb[:], gcnt[:])
            nc.gpsimd.load_library(mlp_lib)
            tc.strict_bb_all_engine_barrier()

            for _ in range(2):
                g0 = gpool.tile([P, TBMAX * ROWP], BF, tag="gath")
                nc.vector.memset(g0[:], 0.0)
            creg = nc.gpsimd.alloc_register("gcnt_reg")

            pool_ps = pspool1.tile([G, F], mybir.dt.float32, tag="pool")

            for layer in (1, 2, 3):
                kt = 1 if layer == 1 else KT
                wt_sb = []
                for k in range(kt):
                    w = wpool.tile([P, ROW], BF, tag=f"wt{k}")
                    nc.sync.dma_start(w[:], wts[layer - 1][k * P:(k + 1) * P, :])
                    wt_sb.append(w)
                brep_sb = wpool.tile([P, F], BF, tag="brep")
                nc.sync.dma_start(brep_sb[:], breps[layer - 1][:])

                # ---- phase A: dense + write hrow_own ----
                for b in range(NB):
                    hlin_ps = pspool.tile([P, ROW], mybir.dt.float32, tag="mm")
                    for k in range(kt):
                        lt = pool.tile([P, P], BF, tag="lhsT")
                        if layer == 1:
                            nc.scalar.dma_start(lt[:], xT[:, b * P:(b + 1) * P])
                        else:
                            nc.scalar.dma_start(
                                lt[:],
                                hT_own[layer - 2][k * P:(k + 1) * P,
                                                  b * P:(b + 1) * P])
                        nc.tensor.matmul(hlin_ps[:], lhsT=lt[:], rhs=wt_sb[k][:],
                                         start=(k == 0), stop=(k == kt - 1))
                    hrow_sb = pool.tile([P, ROW], BF, tag="hrow")
                    nc.scalar.activation(hrow_sb[:], hlin_ps[:], AF.Copy)
                    nc.sync.dma_start(hrow_own[b * P:(b + 1) * P, :ROW],
                                      hrow_sb[:])

                # ---- phase B: AllGather rows ----
                if sim_single:
                    nc.sync.dma_start(hrow_full[:NSHP, :], hrow_own[:])
                else:
                    nc.gpsimd.collective_compute(
                        "AllGather", ALU.bypass,
                        ins=[hrow_own[:]], outs=[hrow_full[:]],
                        replica_groups=[list(range(NCORES))],
                    )

                # ---- phase C: edge phase ----
                t0 = 0
                call_i = 0
                for b in range(NB):
                    Tb = TBSUM[b]
                    # gather: chunked calls per half; per-core actual edge
                    # counts via register (trailing -1 idxs are skipped)
                    gath = gpool.tile([P, TBMAX * ROWP], BF, tag="gath")
                    goff = 0
                    for half in (0, 1):
                        Tg = TBS[b][half]
                        if Tg == 0:
                            continue
                        if "gather" in ABLATE:
                            goff += Tg
                            call_i += (Tg + 7) // 8
                            continue
                        # dma_gather crashes above 1024 idxs/call: chunk <=8
                        done = 0
                        while done < Tg:
                            ck = min(8, Tg - done)
                            o = goff + done
                            if "notrunc" not in ABLATE:
                                nc.gpsimd.reg_load(
                                    creg, gcnt_sb[0:1, call_i:call_i + 1])
                            nc.gpsimd.dma_gather(
                                out_ap=gath[:, o * ROWP:(o + ck) * ROWP]
                                    .rearrange("p (t e) -> p t e", e=ROWP),
                                in_ap=hrow_full[half * HALF:(half + 1) * HALF, :],
                                idxs_ap=idx16_sb[:, 8 * (t0 + o):8 * (t0 + o + ck)],
                                num_idxs=ck * P,
                                num_idxs_reg=(ck * P if "notrunc" in ABLATE
                                              else creg),
                                elem_size=ROWP,
                            )
                            done += ck
                            call_i += 1
                        goff += Tg

                    # one-hot operands (independent of gathered data):
                    # ptall (dst-on-partition) streamed as fp8 from host;
                    # pmat (dst-on-free) built per tile at 4x on DVE
                    ptall = gpool.tile([P, Tb * P], F8, tag="ptall")
                    if "ptload" not in ABLATE:
                        nc.scalar.dma_start(
                            ptall[:], ptall_in[:, t0 * P:(t0 + Tb) * P])
                    pmat = gpool.tile([P, Tb * P], BF, tag="pmat")
                    if "pbuild" not in ABLATE:
                        for t in range(Tb):
                            nc.vector.tensor_scalar(
                                out=pmat[:, t * P:(t + 1) * P], in0=iota_sb[:],
                                scalar1=dstloc_sb[:, t0 + t:t0 + t + 1],
                                scalar2=None, op0=ALU.is_equal)

                    # e_dst expansion via one-hot matmuls
                    edb = pool.tile([P, 4], BF, tag="edb")
                    nc.scalar.dma_start(
                        edb[:], hrow_own[b * P:(b + 1) * P, F + 4:F + 8])
                    edst_ps = pspool.tile([P, 4 * Tb], mybir.dt.float32,
                                          tag="edst")
                    if "edstmm" not in ABLATE:
                        for t in range(Tb):
                            nc.tensor.matmul(edst_ps[:, 4 * t:4 * t + 4],
                                             lhsT=ptall[:, t * P:(t + 1) * P],
                                             rhs=edb[:], start=True, stop=True)

                    # ex = exp(leakyrelu(e_src + e_dst))
                    lg = pool.tile([P, 4 * Tb], BF, tag="lg")
                    nc.vector.tensor_tensor(
                        out=lg[:].rearrange("p (t f) -> p t f", f=4),
                        in0=gath[:, :Tb * ROWP]
                            .rearrange("p (t e) -> p t e", e=ROWP)
                            [:, :, F:F + 4],
                        in1=edst_ps[:].rearrange("p (t f) -> p t f", f=4),
                        op=ALU.add)
                    lr = pool.tile([P, 4 * Tb], BF, tag="lr")
                    nc.vector.scalar_tensor_tensor(
                        out=lr[:], in0=lg[:], scalar=0.2, in1=lg[:],
                        op0=ALU.mult, op1=ALU.max)
                    ex = pool.tile([P, 4 * Tb], BF, tag="ex")
                    if "exp" not in ABLATE:
                        nc.scalar.activation(ex[:], lr[:], AF.Exp)

                    # msg = [h*ex | ex] per tile
                    msg = gpool.tile([P, Tb * (F + 4)], BF, tag="msg")
                    if "muls" not in ABLATE:
                        # c-major feature layout keeps every operand's last AP
                        # dim packed (stride 1) -> DVE 2x mode
                        nc.vector.tensor_tensor(
                            out=msg[:].rearrange("p (t e) -> p t e", e=F + 4)
                                [:, :, 0:F].rearrange("p t (c h) -> p t c h",
                                                      h=H),
                            in0=gath[:, :Tb * ROWP]
                                .rearrange("p (t e) -> p t e", e=ROWP)
                                [:, :, 0:F].rearrange("p t (c h) -> p t c h",
                                                      h=H),
                            in1=ex[:].rearrange("p (t h) -> p t h", h=H)
                                .unsqueeze(2).broadcast_to([P, Tb, HID, H]),
                            op=ALU.mult)
                        nc.vector.tensor_copy(
                            msg[:].rearrange("p (t e) -> p t e", e=F + 4)
                                [:, :, F:F + 4],
                            ex[:].rearrange("p (t h) -> p t h", h=H))

                    # scatter-accumulate [numer | denom] per dst
                    numer_ps = pspool.tile([P, F + 4], mybir.dt.float32,
                                           tag="mm")
                    if "scatter" not in ABLATE:
                        for t in range(Tb):
                            nc.tensor.matmul(
                                numer_ps[:],
                                lhsT=pmat[:, t * P:(t + 1) * P],
                                rhs=msg[:, t * (F + 4):(t + 1) * (F + 4)],
                                start=(t == 0), stop=(t == Tb - 1))

                    # epilogue: y = numer/denom + b
                    dsum = pool.tile([P, H], F32, tag="dsum")
                    # guard pad nodes (zero in-degree): denom=0 -> inf -> NaN
                    nc.vector.tensor_scalar_max(dsum[:], numer_ps[:, F:F + 4],
                                                1e-12)
                    rec = pool.tile([P, H], F32, tag="rec")
                    nc.vector.reciprocal(rec[:], dsum[:])
                    y = pool.tile([P, F], BF, tag="y")
                    nc.vector.tensor_tensor(
                        out=y[:].rearrange("p (c h) -> p c h", h=H),
                        in0=numer_ps[:, 0:F].rearrange("p (c h) -> p c h",
                                                       h=H),
                        in1=rec[:].unsqueeze(1).broadcast_to([P, HID, H]),
                        op=ALU.mult)
                    nc.vector.tensor_tensor(out=y[:], in0=y[:], in1=brep_sb[:],
                                            op=ALU.add)
                    if layer < 3:
                        # ELU: relu(y) + exp(min(y,0)) - 1
                        mn = pool.tile([P, F], BF, tag="mn")
                        nc.vector.tensor_scalar_min(mn[:], y[:], 0.0)
                        eu = pool.tile([P, F], BF, tag="eu")
                        nc.scalar.activation(eu[:], mn[:], AF.Exp)
                        rl = pool.tile([P, F], BF, tag="rl")
                        nc.scalar.activation(rl[:], y[:], AF.Relu)
                        hv = pool.tile([P, F], BF, tag="hv")
                        nc.vector.scalar_tensor_tensor(
                            out=hv[:], in0=eu[:], scalar=-1.0, in1=rl[:],
                            op0=ALU.add, op1=ALU.add)
                        # transpose -> hT_own for next layer's dense phase
                        for k in range(KT):
                            tp = pspool.tile([P, P], BF, tag="tp")
                            nc.tensor.transpose(tp[:], hv[:, k * P:(k + 1) * P],
                                                identb_sb[:])
                            tps = pool.tile([P, P], BF, tag="tps")
                            nc.scalar.activation(tps[:], tp[:], AF.Copy)
                            nc.sync.dma_start(
                                hT_own[layer - 1][k * P:(k + 1) * P,
                                                  b * P:(b + 1) * P], tps[:])
                    else:
                        # pooling accumulation
                        bmat = pool.tile([P, G], BF, tag="bmat")
                        nc.vector.tensor_scalar(
                            out=bmat[:], in0=iota_sb[:, :G],
                            scalar1=batchloc_sb[:, b:b + 1],
                            scalar2=None, op0=ALU.is_equal)
                        nc.tensor.matmul(pool_ps[:], lhsT=bmat[:], rhs=y[:],
                                         start=(b == 0), stop=(b == NB - 1))
                    t0 += Tb
                # fence between layers: hrow/hT buffers are reused
                tc.strict_bb_all_engine_barrier()

            # ---- final: pool -> AllReduce -> mean -> linear ----
            pool_sb = pool.tile([G, F], F32, tag="poolsb")
            nc.vector.tensor_copy(pool_sb[:], pool_ps[:])
            nc.sync.dma_start(pool_own[:], pool_sb[:])
            if sim_single:
                nc.sync.dma_start(pool_full[:], pool_own[:])
            else:
                nc.gpsimd.collective_compute(
                    "AllReduce", ALU.add,
                    ins=[pool_own[:]], outs=[pool_full[:]],
                    replica_groups=[list(range(NCORES))],
                )
            invcnt_sb = cpool.tile([G, 1], F32)
            nc.sync.dma_start(invcnt_sb[:], invcnt[:])
            wlt_sb = []
            for k in range(KT):
                w = cpool.tile([P, OUTF], F32)
                nc.sync.dma_start(w[:], wlt[k * P:(k + 1) * P, :])
                wlt_sb.append(w)
            blrep_sb = cpool.tile([G, OUTF], F32)
            nc.sync.dma_start(blrep_sb[:], blrep[:])

            pooled = pool.tile([G, F], F32, tag="pooled")
            nc.sync.dma_start(pooled[:], pool_full[:])
            mean = pool.tile([G, F], F32, tag="mean")
            nc.vector.tensor_scalar_mul(mean[:], pooled[:], invcnt_sb[:])
            fin_ps = pspool.tile([G, OUTF], mybir.dt.float32, tag="tp")
            for k in range(KT):
                ptp = pspool.tile([P, G], mybir.dt.float32, tag="tp")
                nc.tensor.transpose(ptp[:], mean[:, k * P:(k + 1) * P],
                                    identf_sb[:])
                ptps = pool.tile([P, G], F32, tag="ptps")
                nc.vector.tensor_copy(ptps[:], ptp[:])
                nc.tensor.matmul(fin_ps[:], lhsT=ptps[:], rhs=wlt_sb[k][:],
                                 start=(k == 0), stop=(k == KT - 1))
            outv = pool.tile([G, OUTF], F32, tag="outv")
            nc.vector.tensor_tensor(out=outv[:], in0=fin_ps[:], in1=blrep_sb[:],
                                    op=ALU.add)
            nc.sync.dma_start(out_ext[:], outv[:])

    nc.compile()
    return nc


def preprocess(x, edge_index, batch, params, cfg=CFG):
    """Host-side index preprocessing + param packing -> (TBS, in_maps)."""
    N, G, IN, HID, H, OUTF = (cfg["N"], cfg["G"], cfg["IN"], cfg["HID"],
                              cfg["H"], cfg["OUT"])
    F = H * HID
    NSH = N // NCORES
    NB = (NSH + P - 1) // P
    NSHP = NB * P

    HALF = NCORES * NSHP // 2
    src = np.concatenate([np.asarray(edge_index[0]), np.arange(N)]).astype(np.int64)
    dst = np.concatenate([np.asarray(edge_index[1]), np.arange(N)]).astype(np.int64)
    batch = np.asarray(batch).astype(np.int64)

    def remap(nodes):
        return (nodes // NSH) * NSHP + nodes % NSH

    core_of = dst // NSH
    tiles_lo = np.zeros((NCORES, NB), np.int64)
    tiles_hi = np.zeros((NCORES, NB), np.int64)
    per_core = []
    for c in range(NCORES):
        m = core_of == c
        s_c, d_c = remap(src[m]), dst[m] - c * NSH
        # sort by (block, half, dst) so each block is lo-group then hi-group
        half_c = (s_c >= HALF).astype(np.int64)
        blk = d_c // P
        order = np.lexsort((d_c, half_c, blk))
        s_c, d_c, half_c, blk = s_c[order], d_c[order], half_c[order], blk[order]
        cnt_lo = np.bincount(blk[half_c == 0], minlength=NB)
        cnt_hi = np.bincount(blk[half_c == 1], minlength=NB)
        tiles_lo[c] = (cnt_lo + P - 1) // P
        tiles_hi[c] = (cnt_hi + P - 1) // P
        per_core.append((s_c, d_c, half_c, blk, cnt_lo, cnt_hi))
    TBS = [(int(max(tiles_lo[:, b].max(), 1)), int(tiles_hi[:, b].max()))
           for b in range(NB)]
    TBSUM = [lo + hi for lo, hi in TBS]
    TT = sum(TBSUM)
    tb0 = np.cumsum([0] + TBSUM[:-1])
    tbhi0 = [tb0[b] + TBS[b][0] for b in range(NB)]  # first hi tile per block

    W = {k: np.asarray(v, np.float64) for k, v in params.items()}
    # c-major feature permutation: new position i holds original feature
    # (i%H)*HID + i//H.  Keeps DVE last-dims packed in the edge phase.
    CM = np.array([(i % H) * HID + i // H for i in range(F)])
    wt_aug = {}
    for l in (1, 2, 3):
        Wl = W[f"W{l}"]
        asrc, adst = W[f"a_src{l}"], W[f"a_dst{l}"]
        Ablk_s = np.zeros((F, H))
        Ablk_d = np.zeros((F, H))
        for h in range(H):
            Ablk_s[h * HID:(h + 1) * HID, h] = asrc[h]
            Ablk_d[h * HID:(h + 1) * HID, h] = adst[h]
        wa = np.concatenate([Wl.T[:, CM], Wl.T @ Ablk_s, Wl.T @ Ablk_d], axis=1)
        if l > 1:
            wa = wa[CM, :]        # input rows follow prev layer's layout
        wt_aug[l] = wa.astype(BF_NP)

    counts = np.bincount(batch, minlength=G).astype(np.float64)
    invcnt = (1.0 / np.maximum(counts, 1.0)).astype(np.float32)[:, None]
    iota = np.tile(np.arange(P, dtype=np.float32), (P, 1))

    in_maps = []
    xarr = np.asarray(x)
    for c in range(NCORES):
        s_c, d_c, half_c, blk, cnt_lo, cnt_hi = per_core[c]
        # slot index within the (block, half) group
        grp_key = blk * 2 + half_c
        grp_cnt = np.bincount(grp_key, minlength=2 * NB)
        grp_start = np.concatenate([[0], np.cumsum(grp_cnt)[:-1]])
        pos_in_grp = np.arange(len(d_c)) - grp_start[grp_key]
        grp_t0 = np.where(half_c == 0, tb0[blk], np.asarray(tbhi0)[blk])
        t_idx = (grp_t0 + pos_in_grp // P).astype(np.int64)
        p_idx = (pos_in_grp % P).astype(np.int64)

        dstloc = np.full((P, TT), -1.0, np.float32)
        dstloc[p_idx, t_idx] = (d_c - blk * P).astype(np.float32)
        dstrow = np.ascontiguousarray(dstloc.T).reshape(TT * P)
        ptall8 = np.zeros((P, TT * P), F8_NP)
        cols = np.nonzero(dstrow >= 0)[0]
        ptall8[dstrow[cols].astype(np.int64), cols] = F8_NP(1.0)

        # int16 wrapped gather indices: slot j of tile t -> column 8*t + j//16,
        # partitions p with p%16 == j%16 (replicated across the 8 groups).
        # Pads are -1 (skipped by the DGE); per-call valid counts in gcnt.
        trunc = "notrunc" not in ABLATE
        idxflat = np.full(TT * P, -1 if trunc else 0, np.int16)
        idxflat[t_idx * P + p_idx] = (s_c - half_c * HALF).astype(np.int16)
        gcnt = []
        for b in range(NB):
            for half in (0, 1):
                Tg = TBS[b][half]
                if Tg == 0:
                    continue
                cnt_grp = int((cnt_lo if half == 0 else cnt_hi)[b])
                gs = (tb0[b] if half == 0 else tbhi0[b]) * P
                done = 0
                while done < Tg:
                    ck = min(8, Tg - done)
                    if trunc:
                        c_call = max(0, min(cnt_grp - done * P, ck * P))
                        if c_call == 0:
                            idxflat[gs + done * P] = 0  # keep >=1 valid idx
                            c_call = 1
                    else:
                        c_call = ck * P
                    gcnt.append(c_call)
                    done += ck
        gcnt = np.asarray(gcnt, np.int32)[None, :]
        idx16 = np.ascontiguousarray(
            np.tile(idxflat.reshape(TT * 8, 16).T, (8, 1))).astype(np.int16)

        batchloc = np.full(NSHP, -1.0, np.float32)
        batchloc[:NSH] = batch[c * NSH:(c + 1) * NSH]
        batchloc = np.ascontiguousarray(batchloc.reshape(NB, P).T)

        xT_own = np.zeros((IN, NSHP), BF_NP)
        xT_own[:, :NSH] = xarr[c * NSH:(c + 1) * NSH].astype(BF_NP).T

        in_maps.append(dict(
            xT=xT_own, idx16=idx16, ptall8=ptall8,
            dstloc=dstloc, gcnt=gcnt,
            iotap=np.arange(P, dtype=np.float32)[:, None],
            batchloc=batchloc, iota=iota.astype(BF_NP),
            identb=np.eye(P, dtype=BF_NP),
            identf=np.eye(G, dtype=np.float32),
            wt1=wt_aug[1], wt2=wt_aug[2], wt3=wt_aug[3],
            brep1=np.tile(W["b1"][CM].astype(BF_NP), (P, 1)),
            brep2=np.tile(W["b2"][CM].astype(BF_NP), (P, 1)),
            brep3=np.tile(W["b3"][CM].astype(BF_NP), (P, 1)),
            wlt=np.ascontiguousarray(W["Wl"].T[CM, :].astype(np.float32)),
            blrep=np.tile(W["bl"].astype(np.float32), (G, 1)),
            invcnt=invcnt,
        ))
    return TBS, in_maps


def kernel(**inputs):
    x = inputs.pop("x")
    edge_index = inputs.pop("edge_index")
    batch = inputs.pop("batch")
    TBS, in_maps = preprocess(x, edge_index, batch, inputs)
    nc = build_program(TBS)
    res = run_bass_kernel_spmd(nc, in_maps, list(range(NCORES)))
    return np.asarray(res.results[0]["out"], np.float32)
